# revision 36
# baseline (speedup 1.0000x reference)
"""Causal single-head attention (B=4, S=2048, D=1024, E=1024) on 8 TRN2 cores.

Sharding: 2 cores per batch (core = 2b + par). Core slot j owns global query
block i = 2j + par (128 rows) - interleaving balances the causal triangle.
Slots are processed in PAIRS so the scores matmul streams 256 query columns
per stationary K-block.

Scores are computed TRANSPOSED: st[key, q] = sum_e kt[e,key] * qt[e,q], in
fp8e4m3 DoubleRow perf mode (256-deep contraction pairs, ~2x fp16 rate).
The K and Q projections run FULLY as fp8e4 DoubleRow matmuls, and the
output ships as fp16 (measured max-metric rel err 1.669e-2 vs the 2e-2
gate; the max-metric is not monotone in l2 noise - deterministic
cancellation at the max point). V projection and P@V stay fp16 - V-path
noise transfers ~1:1 to the output and breaks the budget.
Softmax denominators come from an extra ones-column appended to V, so
denom[q] = P @ ones shares the P@V stationary. Causal boundary is additive
fp32 masks (per-parity DATA; program is SPMD-uniform).

The K/V projections + 2-core AllGathers for rep r+1 are software-pipelined
into rep r (parity double-buffered exchange buffers), so every collective
(~10-25us each, serialized CC queue) retires a full rep before its consumer
unpacks it. Collective-input writes get a dedicated HWDGE ring: their
completion gates the AllGather doorbell and must never queue behind bulk
loads; the SWDGE ring (~2us fixed cost per dma_start) only carries unpacks
whose AllGather-waits cannot head-block anything.

Sustained-load measurement (PE SW-throttled to ~1.95-2.1GHz): 131.7us.
"""

import sys

if "/opt/trn_rl_repo" not in sys.path:
    sys.path.insert(0, "/opt/trn_rl_repo")

import numpy as np

B, S, D, E = 4, 2048, 1024, 1024
NCORES = 8
NBLK = 8          # query slots per core (128 rows each)
P = 128
SH = S // 2       # keys projected per core
NPAIR = NBLK // 2
SCALE = 1.0 / 32.0  # 1/sqrt(E)
VW = 1032         # padded per-chunk width of v_sb (1024 e + ones col + pad)

_cache = {}


def _build_program(reps=1):
    import concourse.bass as bass
    import concourse.tile as tile
    from concourse import bacc, mybir
    from concourse.bass import ts, ds
    from contextlib import ExitStack

    dt = mybir.dt
    AF = mybir.ActivationFunctionType
    f16 = dt.float16
    f8 = dt.float8e4    # e4m3: DoubleRow perf mode requires fp8e4/fp8e5
    DR = mybir.MatmulPerfMode.DoubleRow

    nc = bacc.Bacc(
        "TRN2", target_bir_lowering=False, debug=False, enable_asserts=False,
        num_devices=NCORES,
    )

    xt_q = nc.dram_tensor("xt_q", [D, NBLK * P], f16, kind="ExternalInput").ap()
    xt_kv = nc.dram_tensor("xt_kv", [D, SH], f16, kind="ExternalInput").ap()
    wq = nc.dram_tensor("wq", [D, E], f16, kind="ExternalInput").ap()
    wk = nc.dram_tensor("wk", [D, E], f16, kind="ExternalInput").ap()
    wv = nc.dram_tensor("wv", [D, E], f16, kind="ExternalInput").ap()
    maskd = nc.dram_tensor("mask", [2, P, P], dt.float32, kind="ExternalInput").ap()
    # fp16 output: adds ~5e-4 relative noise (negligible vs the 1.67e-2
    # error floor) but halves the 4MB/rep output-DMA volume whose drain
    # gates the out_t pool release, and halves epilogue copy bytes
    out = nc.dram_tensor("out", [NBLK, P, E], f16, kind="ExternalOutput").ap()

    # pair-exchange buffers: K in two half-AllGathers (fired as each K-proj
    # half completes), V in two. The collective queue serializes AGs at
    # ~10-15us each, so the first must be triggerable by ~25us.
    KG = SH // 2      # keys per K group
    VG = SH // 2      # keys per V group
    # double-buffered by rep parity: rep r's attention reads buffer r%2
    # while rep r+1's AllGathers fill (r+1)%2
    cc_ink = [[nc.dram_tensor(f"cc_ink{b}_{g}", [E, KG], f8).ap()
               for g in range(2)] for b in range(2)]
    cc_outk = [[nc.dram_tensor(f"cc_outk{b}_{g}", [2, E, KG], f8).ap()
                for g in range(2)] for b in range(2)]
    cc_inv = [[nc.dram_tensor(f"cc_inv{b}_{g}", [VG, E], f16).ap()
               for g in range(2)] for b in range(2)]
    cc_outv = [[nc.dram_tensor(f"cc_outv{b}_{g}", [2, VG, E], f16).ap()
                for g in range(2)] for b in range(2)]
    GROUPS = [[0, 1], [2, 3], [4, 5], [6, 7]]

    DC = D // P   # 8 contraction chunks
    EC = E // P   # 8 e chunks

    with tile.TileContext(nc) as tc, ExitStack() as ctx:
        consts = ctx.enter_context(tc.tile_pool(name="consts", bufs=1))

        wk_sb = consts.tile([P, DC, E], f16, tag="wk")
        wv_sb = consts.tile([P, DC, E], f16, tag="wv")
        wq_sb = consts.tile([P, DC, E], f16, tag="wq")
        xkv_sb = consts.tile([P, DC, SH], f16, tag="xkv")
        xq_sb = consts.tile([P, DC, NBLK * P], f16, tag="xq")
        kt_sb = consts.tile([P, EC, S], f8, tag="kt")
        qt_sb = consts.tile([P, EC, NBLK * P], f8, tag="qt")
        v_sb = consts.tile([P, S // P, VW], f16, tag="v")
        mask_sb = consts.tile([P, 2, P], dt.float32, tag="mask")
        # e4m3 copies of the last two D-chunks of Wk / x_kv: one quarter of
        # the K-projection contraction runs as a DoubleRow fp8 matmul
        # (error budget: rel err ~1.57e-2 -> ~1.76e-2, gate 2e-2)
        wk8_sb = consts.tile([P, 8, E], f8, tag="wk8")
        xkv8_sb = consts.tile([P, 8, SH], f8, tag="xkv8")
        wq8_sb = consts.tile([P, 8, E], f8, tag="wq8")
        xq8_sb = consts.tile([P, 8, NBLK * P], f8, tag="xq8")

        # All loads on the sync ring, ordered by first use: the scalar ring
        # is reserved for collective-input writes so they never queue behind
        # megabytes of loads (their completion gates the AllGather doorbell).
        # SWDGE (gpsimd) only gets the V-unpacks, whose AllGather-waits must
        # not head-block anything.
        for dc in range(DC):
            nc.sync.dma_start(wk_sb[:, dc, :], wk[dc * P : (dc + 1) * P, :])
            nc.sync.dma_start(xkv_sb[:, dc, :], xt_kv[dc * P : (dc + 1) * P, :])
        for ko in range(2):
            nc.sync.dma_start(mask_sb[:, ko, :], maskd[ko])
        for dc in range(DC):
            nc.sync.dma_start(wv_sb[:, dc, :], wv[dc * P : (dc + 1) * P, :])
        for dc in range(DC):
            nc.sync.dma_start(xq_sb[:, dc, :], xt_q[dc * P : (dc + 1) * P, :])
            nc.sync.dma_start(wq_sb[:, dc, :], wq[dc * P : (dc + 1) * P, :])
        # softmax-denominator ones column of every V chunk
        nc.vector.memset(v_sb[:, :, ds(E, 1)], 1.0)
        for i in range(8):
            nc.vector.tensor_copy(wk8_sb[:, i, :], wk_sb[:, i, :])
            nc.scalar.copy(xkv8_sb[:, i, :], xkv_sb[:, i, :])
        for i in range(8):
            nc.vector.tensor_copy(wq8_sb[:, i, :], wq_sb[:, i, :])
            nc.scalar.copy(xq8_sb[:, i, :], xq_sb[:, i, :])

        def phase1(tc, _rep, reps):
            # Phase 1 of rep r: unpack rep r's K/V (AllGathers already done
            # during rep r-1), project+ship rep r+1's K/V, project rep r's Q.
            with (
                tc.tile_pool(name="proj_ps", bufs=6, space="PSUM") as pp,
                tc.tile_pool(name="stage", bufs=8) as stg,
            ):
                def k_half(tn, pb):
                    # K^T [e, local keys tn*512..+512] in fp8.
                    # D-chunks 0-5 in fp16; chunks 6,7 fused into one
                    # DoubleRow fp8 matmul (256-deep) into the same PSUM.
                    for ec in range(EC):
                        ps = pp.tile([P, 512], dt.float32, tag="proj")
                        for g8 in range(4):
                            nc.tensor.matmul(
                                ps[:],
                                wk8_sb[:, 2 * g8 : 2 * g8 + 2, ts(ec, P)],
                                xkv8_sb[:, 2 * g8 : 2 * g8 + 2, ts(tn, 512)],
                                start=(g8 == 0),
                                stop=(g8 == 3),
                                perf_mode=DR,
                            )
                        st = stg.tile([P, 512], f8, name="stk", tag="stk")
                        (nc.scalar.copy if ec % 2 else nc.vector.tensor_copy)(st[:], ps[:])
                        nc.scalar.dma_start(cc_ink[pb][tn][ts(ec, P), :], st[:])
                    nc.gpsimd.collective_compute(
                        "AllGather", mybir.AluOpType.bypass,
                        replica_groups=GROUPS,
                        ins=[cc_ink[pb][tn]], outs=[cc_outk[pb][tn][:]],
                    )

                def v_half(h, pb):
                    # V [local keys h*512..+512, e] in fp16
                    for tcc in range(4 * h, 4 * h + 4):
                        for en in range(2):
                            ps = pp.tile([P, 512], dt.float32, tag="proj")
                            for dc in range(DC):
                                nc.tensor.matmul(
                                    ps[:],
                                    xkv_sb[:, dc, ts(tcc, P)],
                                    wv_sb[:, dc, ts(en, 512)],
                                    start=(dc == 0),
                                    stop=(dc == DC - 1),
                                )
                            st = stg.tile([P, 512], f16, tag="st")
                            (nc.scalar.copy if en else nc.vector.tensor_copy)(st[:], ps[:])
                            nc.scalar.dma_start(
                                cc_inv[pb][h][ts(tcc - 4 * h, P), ts(en, 512)], st[:]
                            )
                    nc.gpsimd.collective_compute(
                        "AllGather", mybir.AluOpType.bypass,
                        replica_groups=GROUPS,
                        ins=[cc_inv[pb][h]], outs=[cc_outv[pb][h][:]],
                    )

                def kv_proj_and_ag(pb):
                    k_half(0, pb)
                    v_half(0, pb)
                    k_half(1, pb)
                    v_half(1, pb)

                def unpack_all(pb):
                    # gathered K/V of THIS rep: the AllGathers completed
                    # during the previous rep, so these are plain DMAs.
                    for tn in range(2):
                        for r in range(2):
                            nc.sync.dma_start(
                                kt_sb[:, :, ds(r * SH + tn * KG, KG)],
                                cc_outk[pb][tn][r].rearrange("(ec p) k -> p ec k", p=P),
                            )
                    for h in range(2):
                        for r in range(2):
                            eng = nc.sync if (h == 1 and r == 0) else nc.gpsimd
                            eng.dma_start(
                                v_sb[:, r * 8 + 4 * h : r * 8 + 4 * h + 4, ds(0, E)],
                                cc_outv[pb][h][r].rearrange("(c p) e -> p c e", p=P),
                            )

                if _rep == 0:
                    # prologue: ship rep 0's own K/V
                    kv_proj_and_ag(0)
                    unpack_all(0)
                    if reps > 1:
                        kv_proj_and_ag(1)
                else:
                    # next rep's projections lead; this rep's unpacks (plain
                    # DMAs into the other parity's buffers, AllGathers done
                    # last rep) slot in behind so their attention-end WAR
                    # waits don't head the phase
                    if _rep + 1 < reps:
                        kv_proj_and_ag((_rep + 1) % 2)
                    unpack_all(_rep % 2)

                # Q^T [e, q] straight into SBUF (qn outer: attention pairs
                # 0/1 only need the first 512 q columns)
                for qn in range(2):
                    for ec in range(EC):
                        ps = pp.tile([P, 512], dt.float32, tag="proj")
                        for g8 in range(4):
                            nc.tensor.matmul(
                                ps[:],
                                wq8_sb[:, 2 * g8 : 2 * g8 + 2, ts(ec, P)],
                                xq8_sb[:, 2 * g8 : 2 * g8 + 2, ts(qn, 512)],
                                start=(g8 == 0),
                                stop=(g8 == 3),
                                perf_mode=DR,
                            )
                        (nc.scalar.copy if (ec + qn) % 2 else nc.vector.tensor_copy)(
                            qt_sb[:, ec, ts(qn, 512)], ps[:]
                        )

        for _rep in range(reps):
            phase1(tc, _rep, reps)
            # ---- Phase 2: attention (scores transposed, slot pairs) ----
            with (
                tc.tile_pool(name="score_ps", bufs=2, space="PSUM") as sp,
                tc.tile_pool(name="out_ps", bufs=1, space="PSUM") as op,
                tc.tile_pool(name="den_ps0", bufs=1, space="PSUM") as dn0,
                tc.tile_pool(name="den_ps1", bufs=1, space="PSUM") as dn1,
                tc.tile_pool(name="pt", bufs=4) as ptp,
                tc.tile_pool(name="work", bufs=3) as wp,
                tc.tile_pool(name="small", bufs=2) as smp,
            ):
                # flat (pair, t') stream, P@V delayed one step and epilogues
                # emitted mid-stream so PE never waits on exp or epilogue
                state = {}

                def _scores(p, tt):
                    shared = tt <= 2 * p
                    width = 256 if shared else 128
                    qoff = 0 if shared else 128
                    pt = ptp.tile([P, 2, 256], f16, name="pt", tag="pt")
                    for ko in range(2):
                        kb = 2 * tt + ko
                        ps_s = sp.tile([P, 512], dt.float32, name="ps_s", tag="ps_s")
                        for g in range(EC // 2):
                            nc.tensor.matmul(
                                ps_s[:, 0:width],
                                kt_sb[:, 2 * g : 2 * g + 2, ts(kb, P)],
                                qt_sb[:, 2 * g : 2 * g + 2, ds(p * 256 + qoff, width)],
                                start=(g == 0),
                                stop=(g == EC // 2 - 1),
                                perf_mode=DR,
                            )
                        if tt >= 2 * p:
                            # diagonal 256-key chunk of slot (tt - 2p)
                            nc.vector.tensor_add(
                                ps_s[:, 0:P], ps_s[:, 0:P], mask_sb[:, ko, :]
                            )
                        nc.scalar.activation(
                            pt[:, ko, ds(qoff, width)], ps_s[:, 0:width],
                            AF.Exp, bias=0.0, scale=SCALE,
                        )
                    return pt

                def _pv(p, tt, pt):
                    ops, dens = state[p]
                    for s in range(2):
                        if s == 0 and tt > 2 * p:
                            continue
                        for ko in range(2):
                            c = 2 * tt + ko
                            stat = pt[:, ko, ts(s, P)]
                            first = tt == 0 and ko == 0
                            fin = tt == (2 * p + s) and ko == 1
                            for en in range(2):
                                nc.tensor.matmul(
                                    ops[s][en][:], stat,
                                    v_sb[:, c, ts(en, 512)],
                                    start=first, stop=fin,
                                )
                            nc.tensor.matmul(
                                dens[s][:, ds(0, 1)], stat,
                                v_sb[:, c, ds(E, 1)],
                                start=first, stop=fin,
                            )

                def _epilogue(p, s):
                    ops, dens = state[p]
                    if s == 1:
                        state.pop(p)
                    recip = smp.tile([P, 1], dt.float32, name="recip", tag="recip")
                    nc.vector.reciprocal(recip[:], dens[s][:, ds(0, 1)])
                    out_t = wp.tile([P, E], f16, name="out_t", tag="out_t")
                    nc.scalar.activation(
                        out_t[:, 0:512], ops[s][0][:], AF.Copy, scale=recip[:],
                    )
                    nc.vector.tensor_scalar_mul(
                        out_t[:, 512:1024], ops[s][1][:], recip[:]
                    )
                    nc.sync.dma_start(out[2 * p + s], out_t[:])

                items = [(p, tt) for p in range(NPAIR) for tt in range(2 * p + 2)]
                pending = None
                for p, tt in items:
                    if tt == 0:
                        state[p] = (
                            [[op.tile([P, 512], dt.float32, name=f"o{s}{en}",
                                      tag=f"o{s}{en}") for en in range(2)]
                             for s in range(2)],
                            [dn0.tile([P, 8], dt.float32, name="den0", tag="den0"),
                             dn1.tile([P, 8], dt.float32, name="den1", tag="den1")],
                        )
                    pt = _scores(p, tt)
                    if pending is not None:
                        pp_, tt_, pt_ = pending
                        _pv(pp_, tt_, pt_)
                        if tt_ == 2 * pp_:
                            _epilogue(pp_, 0)
                        elif tt_ == 2 * pp_ + 1:
                            _epilogue(pp_, 1)
                    pending = (p, tt, pt)
                pp_, tt_, pt_ = pending
                _pv(pp_, tt_, pt_)
                _epilogue(pp_, 1)

    nc.compile()
    return nc


def _get_program(reps=1, **kw):
    key = f"nc{reps}{sorted(kw.items())}"
    if key not in _cache:
        _cache[key] = _build_program(reps=reps, **kw)
    return _cache[key]


def _make_in_maps(x, Wq, Wk, Wv):
    f16 = np.float16
    wq_h = np.ascontiguousarray(Wq.astype(f16))
    wk_h = np.ascontiguousarray(Wk.astype(f16))
    wv_h = np.ascontiguousarray(Wv.astype(f16))

    # additive causal masks for the two diagonal key blocks, per parity.
    # mask[ko][k, q] masks scores^T element (key 2j*128+ko*128+k, q of slot j)
    k_i = np.arange(P)[:, None]
    q_i = np.arange(P)[None, :]
    tri = np.where(k_i <= q_i, 0.0, -1e9).astype(np.float32)
    full = np.full((P, P), -1e9, dtype=np.float32)
    zero = np.zeros((P, P), dtype=np.float32)
    masks = [
        np.stack([tri, full]),   # parity 0
        np.stack([zero, tri]),   # parity 1
    ]

    in_maps = []
    for core in range(NCORES):
        b, par = core // 2, core % 2
        xt = np.ascontiguousarray(x[b].T.astype(f16))  # [D, S]
        blocks = [2 * j + par for j in range(NBLK)]
        xt_q = np.ascontiguousarray(
            xt.reshape(D, S // P, P)[:, blocks, :].reshape(D, NBLK * P)
        )
        xt_kv = np.ascontiguousarray(xt[:, par * SH : (par + 1) * SH])
        in_maps.append(
            {
                "xt_q": xt_q,
                "xt_kv": xt_kv,
                "wq": wq_h,
                "wk": wk_h,
                "wv": wv_h,
                "mask": masks[par],
            }
        )
    return in_maps


def _assemble(results):
    out = np.empty((B, S, E), dtype=np.float32)
    for core in range(NCORES):
        b, par = core // 2, core % 2
        o = results[core]["out"]  # [NBLK, P, E]
        for j in range(NBLK):
            i = 2 * j + par
            out[b, i * P : (i + 1) * P, :] = o[j]
    return out


def run(inputs, trace=False, reps=1):
    from concourse import bass_utils

    x = np.asarray(inputs["x"], dtype=np.float32)
    Wq = np.asarray(inputs["Wq"], dtype=np.float32)
    Wk = np.asarray(inputs["Wk"], dtype=np.float32)
    Wv = np.asarray(inputs["Wv"], dtype=np.float32)

    nc = _get_program(reps=reps)
    in_maps = _make_in_maps(x, Wq, Wk, Wv)
    res = bass_utils.run_bass_kernel_spmd(
        nc, in_maps, core_ids=list(range(NCORES)), trace=trace
    )
    return _assemble(res.results), res


def kernel(**inputs):
    out, _ = run(inputs, trace=False)
    return out


# revision 37
# speedup vs baseline: 1.0140x; 1.0140x over previous
"""Causal single-head attention (B=4, S=2048, D=1024, E=1024) on 8 TRN2 cores.

Sharding: 2 cores per batch (core = 2b + par). Core slot j owns global query
block i = 2j + par (128 rows) - interleaving balances the causal triangle.
Slots are processed in PAIRS so the scores matmul streams 256 query columns
per stationary K-block.

Scores are computed TRANSPOSED: st[key, q] = sum_e kt[e,key] * qt[e,q], in
fp8e4m3 DoubleRow perf mode (256-deep contraction pairs, ~2x fp16 rate).
The K and Q projections run FULLY as fp8e4 DoubleRow matmuls, and the
output ships as fp16 (measured max-metric rel err 1.669e-2 vs the 2e-2
gate; the max-metric is not monotone in l2 noise - deterministic
cancellation at the max point). V projection and P@V stay fp16 - V-path
noise transfers ~1:1 to the output and breaks the budget.
Softmax denominators come from an extra ones-column appended to V, so
denom[q] = P @ ones shares the P@V stationary. Causal boundary is additive
fp32 masks (per-parity DATA; program is SPMD-uniform).

The K/V projections + 2-core AllGathers for rep r+1 are software-pipelined
into rep r (parity double-buffered exchange buffers), so every collective
(~10-25us each, serialized CC queue) retires a full rep before its consumer
unpacks it. Collective-input writes get a dedicated HWDGE ring: their
completion gates the AllGather doorbell and must never queue behind bulk
loads; the SWDGE ring (~2us fixed cost per dma_start) only carries unpacks
whose AllGather-waits cannot head-block anything.

Sustained-load measurement (PE SW-throttled to ~1.95-2.1GHz): 131.7us.
"""

import sys

if "/opt/trn_rl_repo" not in sys.path:
    sys.path.insert(0, "/opt/trn_rl_repo")

import numpy as np

B, S, D, E = 4, 2048, 1024, 1024
NCORES = 8
NBLK = 8          # query slots per core (128 rows each)
P = 128
SH = S // 2       # keys projected per core
NPAIR = NBLK // 2
SCALE = 1.0 / 32.0  # 1/sqrt(E)
VW = 1032         # padded per-chunk width of v_sb (1024 e + ones col + pad)

_cache = {}


def _build_program(reps=1):
    import concourse.bass as bass
    import concourse.tile as tile
    from concourse import bacc, mybir
    from concourse.bass import ts, ds
    from contextlib import ExitStack

    dt = mybir.dt
    AF = mybir.ActivationFunctionType
    f16 = dt.float16
    f8 = dt.float8e4    # e4m3: DoubleRow perf mode requires fp8e4/fp8e5
    DR = mybir.MatmulPerfMode.DoubleRow

    nc = bacc.Bacc(
        "TRN2", target_bir_lowering=False, debug=False, enable_asserts=False,
        num_devices=NCORES,
    )

    xt_q = nc.dram_tensor("xt_q", [D, NBLK * P], f16, kind="ExternalInput").ap()
    xt_kv = nc.dram_tensor("xt_kv", [D, SH], f16, kind="ExternalInput").ap()
    wq = nc.dram_tensor("wq", [D, E], f16, kind="ExternalInput").ap()
    wk = nc.dram_tensor("wk", [D, E], f16, kind="ExternalInput").ap()
    wv = nc.dram_tensor("wv", [D, E], f16, kind="ExternalInput").ap()
    maskd = nc.dram_tensor("mask", [2, P, P], dt.float32, kind="ExternalInput").ap()
    # fp16 output: adds ~5e-4 relative noise (negligible vs the 1.67e-2
    # error floor) but halves the 4MB/rep output-DMA volume whose drain
    # gates the out_t pool release, and halves epilogue copy bytes
    out = nc.dram_tensor("out", [NBLK, P, E], f16, kind="ExternalOutput").ap()

    # pair-exchange buffers: K in two half-AllGathers (fired as each K-proj
    # half completes), V in two. The collective queue serializes AGs at
    # ~10-15us each, so the first must be triggerable by ~25us.
    KG = SH // 2      # keys per K group
    VG = SH // 2      # keys per V group
    # double-buffered by rep parity: rep r's attention reads buffer r%2
    # while rep r+1's AllGathers fill (r+1)%2
    cc_ink = [[nc.dram_tensor(f"cc_ink{b}_{g}", [E, KG], f8).ap()
               for g in range(2)] for b in range(2)]
    cc_outk = [[nc.dram_tensor(f"cc_outk{b}_{g}", [2, E, KG], f8).ap()
                for g in range(2)] for b in range(2)]
    cc_inv = [[nc.dram_tensor(f"cc_inv{b}_{g}", [VG, E], f16).ap()
               for g in range(2)] for b in range(2)]
    cc_outv = [[nc.dram_tensor(f"cc_outv{b}_{g}", [2, VG, E], f16).ap()
                for g in range(2)] for b in range(2)]
    GROUPS = [[0, 1], [2, 3], [4, 5], [6, 7]]

    DC = D // P   # 8 contraction chunks
    EC = E // P   # 8 e chunks

    with tile.TileContext(nc) as tc, ExitStack() as ctx:
        consts = ctx.enter_context(tc.tile_pool(name="consts", bufs=1))

        wk_sb = consts.tile([P, DC, E], f16, tag="wk")
        wv_sb = consts.tile([P, DC, E], f16, tag="wv")
        wq_sb = consts.tile([P, DC, E], f16, tag="wq")
        xkv_sb = consts.tile([P, DC, SH], f16, tag="xkv")
        xq_sb = consts.tile([P, DC, NBLK * P], f16, tag="xq")
        kt_sb = consts.tile([P, EC, S], f8, tag="kt")
        qt_sb = consts.tile([P, EC, NBLK * P], f8, tag="qt")
        v_sb = consts.tile([P, S // P, VW], f16, tag="v")
        mask_sb = consts.tile([P, 2, P], dt.float32, tag="mask")
        # e4m3 copies of the last two D-chunks of Wk / x_kv: one quarter of
        # the K-projection contraction runs as a DoubleRow fp8 matmul
        # (error budget: rel err ~1.57e-2 -> ~1.76e-2, gate 2e-2)
        wk8_sb = consts.tile([P, 8, E], f8, tag="wk8")
        xkv8_sb = consts.tile([P, 8, SH], f8, tag="xkv8")
        wq8_sb = consts.tile([P, 8, E], f8, tag="wq8")
        xq8_sb = consts.tile([P, 8, NBLK * P], f8, tag="xq8")

        # All loads on the sync ring, ordered by first use: the scalar ring
        # is reserved for collective-input writes so they never queue behind
        # megabytes of loads (their completion gates the AllGather doorbell).
        # SWDGE (gpsimd) only gets the V-unpacks, whose AllGather-waits must
        # not head-block anything.
        for dc in range(DC):
            nc.sync.dma_start(wk_sb[:, dc, :], wk[dc * P : (dc + 1) * P, :])
            nc.sync.dma_start(xkv_sb[:, dc, :], xt_kv[dc * P : (dc + 1) * P, :])
        for ko in range(2):
            nc.sync.dma_start(mask_sb[:, ko, :], maskd[ko])
        for dc in range(DC):
            nc.sync.dma_start(wv_sb[:, dc, :], wv[dc * P : (dc + 1) * P, :])
        for dc in range(DC):
            nc.sync.dma_start(xq_sb[:, dc, :], xt_q[dc * P : (dc + 1) * P, :])
            nc.sync.dma_start(wq_sb[:, dc, :], wq[dc * P : (dc + 1) * P, :])
        # softmax-denominator ones column of every V chunk
        nc.vector.memset(v_sb[:, :, ds(E, 1)], 1.0)
        for i in range(8):
            nc.vector.tensor_copy(wk8_sb[:, i, :], wk_sb[:, i, :])
            nc.scalar.copy(xkv8_sb[:, i, :], xkv_sb[:, i, :])
        for i in range(8):
            nc.vector.tensor_copy(wq8_sb[:, i, :], wq_sb[:, i, :])
            nc.scalar.copy(xq8_sb[:, i, :], xq_sb[:, i, :])

        def phase1(tc, _rep, reps):
            # Phase 1 of rep r: unpack rep r's K/V (AllGathers already done
            # during rep r-1), project+ship rep r+1's K/V, project rep r's Q.
            with (
                tc.tile_pool(name="proj_ps", bufs=6, space="PSUM") as pp,
                tc.tile_pool(name="stage", bufs=8) as stg,
            ):
                def k_half(tn, pb):
                    # K^T [e, local keys tn*512..+512] in fp8.
                    # D-chunks 0-5 in fp16; chunks 6,7 fused into one
                    # DoubleRow fp8 matmul (256-deep) into the same PSUM.
                    for ec in range(EC):
                        ps = pp.tile([P, 512], dt.float32, tag="proj")
                        for g8 in range(4):
                            nc.tensor.matmul(
                                ps[:],
                                wk8_sb[:, 2 * g8 : 2 * g8 + 2, ts(ec, P)],
                                xkv8_sb[:, 2 * g8 : 2 * g8 + 2, ts(tn, 512)],
                                start=(g8 == 0),
                                stop=(g8 == 3),
                                perf_mode=DR,
                            )
                        st = stg.tile([P, 512], f8, name="stk", tag="stk")
                        (nc.scalar.copy if ec % 2 else nc.vector.tensor_copy)(st[:], ps[:])
                        nc.scalar.dma_start(cc_ink[pb][tn][ts(ec, P), :], st[:])
                    nc.gpsimd.collective_compute(
                        "AllGather", mybir.AluOpType.bypass,
                        replica_groups=GROUPS,
                        ins=[cc_ink[pb][tn]], outs=[cc_outk[pb][tn][:]],
                    )

                def v_half(h, pb):
                    # V [local keys h*512..+512, e] in fp16
                    for tcc in range(4 * h, 4 * h + 4):
                        for en in range(2):
                            ps = pp.tile([P, 512], dt.float32, tag="proj")
                            for dc in range(DC):
                                nc.tensor.matmul(
                                    ps[:],
                                    xkv_sb[:, dc, ts(tcc, P)],
                                    wv_sb[:, dc, ts(en, 512)],
                                    start=(dc == 0),
                                    stop=(dc == DC - 1),
                                )
                            st = stg.tile([P, 512], f16, tag="st")
                            (nc.scalar.copy if en else nc.vector.tensor_copy)(st[:], ps[:])
                            nc.scalar.dma_start(
                                cc_inv[pb][h][ts(tcc - 4 * h, P), ts(en, 512)], st[:]
                            )
                    nc.gpsimd.collective_compute(
                        "AllGather", mybir.AluOpType.bypass,
                        replica_groups=GROUPS,
                        ins=[cc_inv[pb][h]], outs=[cc_outv[pb][h][:]],
                    )

                def kv_proj_and_ag(pb):
                    k_half(0, pb)
                    v_half(0, pb)
                    k_half(1, pb)
                    v_half(1, pb)

                def unpack_all(pb):
                    # gathered K/V of THIS rep: the AllGathers completed
                    # during the previous rep, so these are plain DMAs.
                    for tn in range(2):
                        for r in range(2):
                            nc.sync.dma_start(
                                kt_sb[:, :, ds(r * SH + tn * KG, KG)],
                                cc_outk[pb][tn][r].rearrange("(ec p) k -> p ec k", p=P),
                            )
                    for h in range(2):
                        for r in range(2):
                            eng = nc.sync if (h == 1 and r == 0) else nc.gpsimd
                            eng.dma_start(
                                v_sb[:, r * 8 + 4 * h : r * 8 + 4 * h + 4, ds(0, E)],
                                cc_outv[pb][h][r].rearrange("(c p) e -> p c e", p=P),
                            )

                if _rep == 0:
                    # prologue: ship rep 0's own K/V
                    kv_proj_and_ag(0)
                    unpack_all(0)
                    if reps > 1:
                        kv_proj_and_ag(1)
                else:
                    # next rep's projections lead; this rep's unpacks (plain
                    # DMAs into the other parity's buffers, AllGathers done
                    # last rep) slot in behind so their attention-end WAR
                    # waits don't head the phase
                    if _rep + 1 < reps:
                        kv_proj_and_ag((_rep + 1) % 2)
                    unpack_all(_rep % 2)

                # Q^T [e, q] straight into SBUF (qn outer: attention pairs
                # 0/1 only need the first 512 q columns)
                for qn in range(2):
                    for ec in range(EC):
                        ps = pp.tile([P, 512], dt.float32, tag="proj")
                        for g8 in range(4):
                            nc.tensor.matmul(
                                ps[:],
                                wq8_sb[:, 2 * g8 : 2 * g8 + 2, ts(ec, P)],
                                xq8_sb[:, 2 * g8 : 2 * g8 + 2, ts(qn, 512)],
                                start=(g8 == 0),
                                stop=(g8 == 3),
                                perf_mode=DR,
                            )
                        (nc.scalar.copy if (ec + qn) % 2 else nc.vector.tensor_copy)(
                            qt_sb[:, ec, ts(qn, 512)], ps[:]
                        )

        for _rep in range(reps):
            phase1(tc, _rep, reps)
            # ---- Phase 2: attention (scores transposed, slot pairs) ----
            with (
                tc.tile_pool(name="score_ps", bufs=2, space="PSUM") as sp,
                tc.tile_pool(name="out_ps", bufs=1, space="PSUM") as op,
                tc.tile_pool(name="den_ps0", bufs=1, space="PSUM") as dn0,
                tc.tile_pool(name="den_ps1", bufs=1, space="PSUM") as dn1,
                tc.tile_pool(name="pt", bufs=4) as ptp,
                tc.tile_pool(name="work", bufs=3) as wp,
                tc.tile_pool(name="small", bufs=2) as smp,
            ):
                # flat (pair, t') stream, P@V delayed one step and epilogues
                # emitted mid-stream so PE never waits on exp or epilogue
                state = {}

                def _scores(p, tt):
                    shared = tt <= 2 * p
                    width = 256 if shared else 128
                    qoff = 0 if shared else 128
                    pt = ptp.tile([P, 2, 256], f16, name="pt", tag="pt")
                    # both ko sub-blocks share one PSUM bank ([P,2,256] fp32)
                    # so the step needs ONE mask-add and ONE exp. ko=0's
                    # start=True clears the whole bank; ko=1 accumulates with
                    # start=False onto the cleared (has_written=0) region.
                    ps_s = sp.tile([P, 2, 256], dt.float32, name="ps_s", tag="ps_s")
                    for ko in range(2):
                        kb = 2 * tt + ko
                        for g in range(EC // 2):
                            nc.tensor.matmul(
                                ps_s[:, ko, 0:width],
                                kt_sb[:, 2 * g : 2 * g + 2, ts(kb, P)],
                                qt_sb[:, 2 * g : 2 * g + 2, ds(p * 256 + qoff, width)],
                                start=(g == 0 and ko == 0),
                                stop=(g == EC // 2 - 1),
                                perf_mode=DR,
                                skip_group_check=True,
                            )
                    if tt >= 2 * p:
                        # diagonal 256-key chunk of slot (tt - 2p)
                        nc.vector.tensor_add(
                            ps_s[:, :, 0:P], ps_s[:, :, 0:P], mask_sb[:, :, :]
                        )
                    nc.scalar.activation(
                        pt[:, :, ds(qoff, width)], ps_s[:, :, 0:width],
                        AF.Exp, bias=0.0, scale=SCALE,
                    )
                    return pt

                def _pv(p, tt, pt):
                    ops, dens = state[p]
                    for s in range(2):
                        if s == 0 and tt > 2 * p:
                            continue
                        for ko in range(2):
                            c = 2 * tt + ko
                            stat = pt[:, ko, ts(s, P)]
                            first = tt == 0 and ko == 0
                            fin = tt == (2 * p + s) and ko == 1
                            for en in range(2):
                                nc.tensor.matmul(
                                    ops[s][en][:], stat,
                                    v_sb[:, c, ts(en, 512)],
                                    start=first, stop=fin,
                                )
                            nc.tensor.matmul(
                                dens[s][:, ds(0, 1)], stat,
                                v_sb[:, c, ds(E, 1)],
                                start=first, stop=fin,
                            )

                def _epilogue(p, s):
                    ops, dens = state[p]
                    if s == 1:
                        state.pop(p)
                    recip = smp.tile([P, 1], dt.float32, name="recip", tag="recip")
                    nc.vector.reciprocal(recip[:], dens[s][:, ds(0, 1)])
                    out_t = wp.tile([P, E], f16, name="out_t", tag="out_t")
                    nc.scalar.activation(
                        out_t[:, 0:512], ops[s][0][:], AF.Copy, scale=recip[:],
                    )
                    nc.vector.tensor_scalar_mul(
                        out_t[:, 512:1024], ops[s][1][:], recip[:]
                    )
                    nc.sync.dma_start(out[2 * p + s], out_t[:])

                items = [(p, tt) for p in range(NPAIR) for tt in range(2 * p + 2)]
                pending = None
                for p, tt in items:
                    if tt == 0:
                        state[p] = (
                            [[op.tile([P, 512], dt.float32, name=f"o{s}{en}",
                                      tag=f"o{s}{en}") for en in range(2)]
                             for s in range(2)],
                            [dn0.tile([P, 8], dt.float32, name="den0", tag="den0"),
                             dn1.tile([P, 8], dt.float32, name="den1", tag="den1")],
                        )
                    pt = _scores(p, tt)
                    if pending is not None:
                        pp_, tt_, pt_ = pending
                        _pv(pp_, tt_, pt_)
                        if tt_ == 2 * pp_:
                            _epilogue(pp_, 0)
                        elif tt_ == 2 * pp_ + 1:
                            _epilogue(pp_, 1)
                    pending = (p, tt, pt)
                pp_, tt_, pt_ = pending
                _pv(pp_, tt_, pt_)
                _epilogue(pp_, 1)

    nc.compile()
    return nc


def _get_program(reps=1, **kw):
    key = f"nc{reps}{sorted(kw.items())}"
    if key not in _cache:
        _cache[key] = _build_program(reps=reps, **kw)
    return _cache[key]


def _make_in_maps(x, Wq, Wk, Wv):
    f16 = np.float16
    wq_h = np.ascontiguousarray(Wq.astype(f16))
    wk_h = np.ascontiguousarray(Wk.astype(f16))
    wv_h = np.ascontiguousarray(Wv.astype(f16))

    # additive causal masks for the two diagonal key blocks, per parity.
    # mask[ko][k, q] masks scores^T element (key 2j*128+ko*128+k, q of slot j)
    k_i = np.arange(P)[:, None]
    q_i = np.arange(P)[None, :]
    tri = np.where(k_i <= q_i, 0.0, -1e9).astype(np.float32)
    full = np.full((P, P), -1e9, dtype=np.float32)
    zero = np.zeros((P, P), dtype=np.float32)
    masks = [
        np.stack([tri, full]),   # parity 0
        np.stack([zero, tri]),   # parity 1
    ]

    in_maps = []
    for core in range(NCORES):
        b, par = core // 2, core % 2
        xt = np.ascontiguousarray(x[b].T.astype(f16))  # [D, S]
        blocks = [2 * j + par for j in range(NBLK)]
        xt_q = np.ascontiguousarray(
            xt.reshape(D, S // P, P)[:, blocks, :].reshape(D, NBLK * P)
        )
        xt_kv = np.ascontiguousarray(xt[:, par * SH : (par + 1) * SH])
        in_maps.append(
            {
                "xt_q": xt_q,
                "xt_kv": xt_kv,
                "wq": wq_h,
                "wk": wk_h,
                "wv": wv_h,
                "mask": masks[par],
            }
        )
    return in_maps


def _assemble(results):
    out = np.empty((B, S, E), dtype=np.float32)
    for core in range(NCORES):
        b, par = core // 2, core % 2
        o = results[core]["out"]  # [NBLK, P, E]
        for j in range(NBLK):
            i = 2 * j + par
            out[b, i * P : (i + 1) * P, :] = o[j]
    return out


def run(inputs, trace=False, reps=1):
    from concourse import bass_utils

    x = np.asarray(inputs["x"], dtype=np.float32)
    Wq = np.asarray(inputs["Wq"], dtype=np.float32)
    Wk = np.asarray(inputs["Wk"], dtype=np.float32)
    Wv = np.asarray(inputs["Wv"], dtype=np.float32)

    nc = _get_program(reps=reps)
    in_maps = _make_in_maps(x, Wq, Wk, Wv)
    res = bass_utils.run_bass_kernel_spmd(
        nc, in_maps, core_ids=list(range(NCORES)), trace=trace
    )
    return _assemble(res.results), res


def kernel(**inputs):
    out, _ = run(inputs, trace=False)
    return out
